# revision 22
# baseline (speedup 1.0000x reference)
"""Trainium2 Bass kernel for the ClassificationNCA problem.

Self-contained: callable as kernel(**inputs) with the full (unsharded)
inputs; shards batch across 8 NeuronCores (2 images/core), runs the
20-step NCA entirely in SBUF, returns softmax(mean-pooled class channels).

v2 design notes (vs the 732us baseline):
- Whole-core stencil: the sobel smooth/diff chain runs once per step on
  all 128 partitions ([img x half x 32ch] blocks) instead of per image,
  halving DVE/Pool stencil time.
- Fire folding: leaky(x*f) = leaky(x)*f for f in {0,1} and per-pixel
  scalars commute through channel matmuls, so multiplying the three
  perception taps (identity/sx/sy strips) by the fire mask up front
  makes every downstream tensor (h1, h2, dx) arrive pre-masked; the
  per-tile fire multiply and its 128-partition broadcast DMA disappear.
- Ones-row bias: state partition-row 29 of each block is constant 1.0;
  after fire masking it carries fire itself, and row 29 of the L1
  s-tap weights holds the (per-step, temporal-augmented) bias.  L1's
  activation is then bias-free and runs as one [128,1024] op per tile.
- L2 runs as a single fp8e4 DoubleRow matmul (K=256 in one pass, 0.5
  cyc/row): h1s is written by the L1 activation as the two K-planes of
  a [128,2,512] fp8 tile.  Weight/activation scaling (x8 into fp8's
  normal range, /64 folded into w3) keeps the math exact up to fp8
  rounding.
- The f32->bf16 shadow copy runs on DVE (TensorCopy 2x_2p mode) and
  engine assignment of every pointwise op class is a knob (ENG dict).
"""
import sys

sys.path.insert(0, "/opt/trn_rl_repo")

import json
import numpy as np
import ml_dtypes

NUM_IMG, NUM_HID, NUM_OUT = 3, 16, 10
NCH = 29            # total channels
HIDDEN = 128
B, H, W = 16, 64, 64
N_CORES = 8
B_LOC = B // N_CORES          # images per core
WP = W + 4                    # padded width (68)
SR = 34                       # strip rows: halo/pad + 32 interior + halo/pad
SFLAT = SR * WP               # 2312
ROWS_PER_TILE = 8
N_RB = 32 // ROWS_PER_TILE    # 4 row blocks per (img, half) strip
NPIX_TILE = ROWS_PER_TILE * W  # 512
CB = 32                       # channel-block partition stride
CL = 2                        # interior column offset
W2SCALE = 8.0                 # w2 prescale into fp8 normal range
H1SCALE = 8.0                 # h1s prescale into fp8 normal range

_MAX_WAITS = 1

# Engine assignment knobs for the flexible op classes.
ENG = {
    "s16": "scalar",     # f32->bf16 shadow copy (vector/scalar/gpsimd)
    "t3": "gpsimd",      # horizontal pair add
    "t4": "gpsimd",      # horizontal [1,2,1] (2nd stage)
    "t1": "vector",      # vertical pair add
    "t2": "vector",      # vertical [1,2,1] (2nd stage)
    "subx": "vector",    # sobel-x diff
    "suby": "vector",    # sobel-y diff
    "mask": "vector",    # fire mask muls (3 per step)
    "halo": "gpsimd",    # halo row sync
    # per-tile index (0..15) -> engine for the L1 activation (scalar/vector:
    # gpsimd cannot read PSUM)
    "l1act": ["scalar"] * 16,
    # per-tile index (0..15) -> engine for the L2 activation (scalar/vector)
    "l2act": ["scalar"] * 16,
    # per-tile index (0..31, two per tile) -> engine for the perc gathers
    "gather": ["vector"] * 32,
}


def _fix_bir_waits(bir_bytes: bytes) -> bytes:
    """walrus codegen allows only one embedded sem-wait per instruction;
    Tile sometimes attaches more.  Move excess waits onto NoOp carriers."""
    bir = json.loads(bir_bytes)
    uid = 0
    for fn in bir["functions"]:
        for blk in fn["blocks"]:
            out = []
            for ins in blk["instructions"]:
                si = ins.get("sync_info")
                waits = (si or {}).get("on_wait") or []
                if len(waits) > _MAX_WAITS:
                    excess = waits[:-_MAX_WAITS]
                    si["on_wait"] = waits[-_MAX_WAITS:]
                    for i in range(0, len(excess), _MAX_WAITS):
                        out.append({
                            "opcode": "NoOp",
                            "name": f"wsplit_{uid}",
                            "engine": ins["engine"],
                            "ins": [],
                            "outs": [],
                            "sync_info": {
                                "on_wait": excess[i:i + _MAX_WAITS],
                                "on_update": [],
                            },
                        })
                        uid += 1
                out.append(ins)
            blk["instructions"] = out
    return json.dumps(bir).encode()


def _host_rng(steps: int):
    """Reproduce the reference's jax threefry random draws exactly (on CPU)."""
    import jax
    cpu = jax.devices("cpu")[0]
    with jax.default_device(cpu):
        import jax.numpy as jnp
        base = jax.random.key(42)
        hid = 0.5 + 0.225 * jax.random.normal(
            jax.random.fold_in(base, 10_000), (B, NUM_HID, H, W),
            dtype=jnp.float32)
        hid = np.asarray(hid)
        fires = np.zeros((max(steps, 1), B, H, W), np.float32)
        for s in range(steps):
            u = jax.random.uniform(jax.random.fold_in(base, s), (B, H, W, 1),
                                   dtype=jnp.float32)
            fires[s] = np.asarray(u < 0.5, np.float32)[..., 0]
    return hid, fires


def _build(steps: int, repeat: int = 1):
    from concourse import mybir
    from concourse.bass import Bass
    from concourse.tile import TileContext

    f32 = mybir.dt.float32
    bf16 = mybir.dt.bfloat16
    fp8 = mybir.dt.float8e4
    LR = mybir.ActivationFunctionType.Lrelu
    DR = mybir.MatmulPerfMode.DoubleRow
    MULT = mybir.AluOpType.mult
    MAX = mybir.AluOpType.max
    ADD = mybir.AluOpType.add
    SUB = mybir.AluOpType.subtract

    nc = Bass(trn_type="TRN2", target_bir_lowering=False)

    s0_d = nc.dram_tensor("s0", [128, SR, WP], bf16, kind="ExternalInput")
    fire_d = nc.dram_tensor("fire", [max(steps, 1), 128, SR, WP],
                            bf16, kind="ExternalInput")
    w1all_d = nc.dram_tensor("w1all", [128, max(steps, 1), 2 * HIDDEN], bf16,
                             kind="ExternalInput")
    w1xy_d = nc.dram_tensor("w1xy", [2 * CB, 2 * HIDDEN], bf16, kind="ExternalInput")
    w2dr_d = nc.dram_tensor("w2dr", [HIDDEN, 2, HIDDEN], fp8, kind="ExternalInput")
    w3T_d = nc.dram_tensor("w3T", [HIDDEN, NCH], bf16, kind="ExternalInput")
    out_d = nc.dram_tensor("sout", [128, SR, WP], bf16, kind="ExternalOutput")

    # interior flat window: every (row 1..32, col 2..65) position lies inside
    LO, HI = WP + 1, SFLAT - WP - 1

    with TileContext(nc) as tc:
        with tc.tile_pool(name="state", bufs=1) as statep, \
             tc.tile_pool(name="wts", bufs=1) as wtsp, \
             tc.tile_pool(name="convs", bufs=1) as convp, \
             tc.tile_pool(name="fire", bufs=2) as firep, \
             tc.tile_pool(name="acts", bufs=8) as actp, \
             tc.tile_pool(name="perc", bufs=10) as percp, \
             tc.tile_pool(name="ph1", bufs=2, space="PSUM") as ph1, \
             tc.tile_pool(name="ph2", bufs=2, space="PSUM") as ph2, \
             tc.tile_pool(name="pdx", bufs=2, space="PSUM") as pdx:

            S = [statep.tile([128, SR, WP], bf16, name=f"S{k}") for k in range(2)]
            s16m = statep.tile([128, SR, WP], bf16, name="s16m")
            SX = statep.tile([128, SR, WP], bf16, name="SX")
            SY = statep.tile([128, SR, WP], bf16, name="SY")
            SXm = statep.tile([128, SR, WP], bf16, name="SXm")
            SYm = statep.tile([128, SR, WP], bf16, name="SYm")
            T1 = convp.tile([128, SFLAT], bf16, name="T1")
            T2 = convp.tile([128, SFLAT], bf16, name="T2")
            T3 = convp.tile([128, SFLAT], bf16, name="T3")
            T4 = convp.tile([128, SFLAT], bf16, name="T4")

            w1all = wtsp.tile([128, max(steps, 1), 2 * HIDDEN], bf16,
                              name="w1all")
            w1xy = wtsp.tile([2 * CB, 2 * HIDDEN], bf16, name="w1xy")
            w2dr = wtsp.tile([HIDDEN, 2, HIDDEN], fp8, name="w2dr")
            w3T = wtsp.tile([HIDDEN, NCH], bf16, name="w3T")

            nc.sync.dma_start(out=S[0][:], in_=s0_d[:])
            nc.sync.dma_start(out=S[1][:], in_=s0_d[:])
            nc.sync.dma_start(out=w1all[:], in_=w1all_d[:])
            nc.sync.dma_start(out=w1xy[:], in_=w1xy_d[:])
            nc.sync.dma_start(out=w2dr[:], in_=w2dr_d[:])
            nc.sync.dma_start(out=w3T[:], in_=w3T_d[:])

            # Zero the stencil temporaries once so windowed writes never
            # leave uninitialized SBUF visible to later full-window reads.
            for tl in (T1, T2, T3, T4):
                nc.vector.memset(tl[:], 0.0)
            for tl in (SX, SY, SXm, SYm, s16m):
                nc.vector.memset(tl[:, :, :].rearrange("p a b -> p (a b)"), 0.0)

            def eng(which):
                return getattr(nc, {"vector": "vector", "scalar": "scalar",
                                    "gpsimd": "gpsimd"}[which])

            def tcopy(which, out, in_):
                if which == "scalar":
                    nc.scalar.copy(out=out, in_=in_)
                else:
                    eng(which).tensor_copy(out=out, in_=in_)

            def tt(which, op, out, in0, in1):
                """2-input elementwise op (TensorScalarPtr is not a legal
                opcode on Pool, so plain TensorTensor everywhere)."""
                eng(which).tensor_tensor(out=out, in0=in0, in1=in1, op=op)

            fts = {}

            def get_ft(g):
                if g not in fts:
                    ftt = firep.tile([128, SR, WP], bf16, name="ft")
                    nc.sync.dma_start(out=ftt[:], in_=fire_d[g % steps])
                    fts[g] = ftt
                return fts[g]

            def emit_chunk(g, c):
                """Stencil + fire masking for row-quarter c (output strip
                rows 1+8c .. 8+8c) of step g, all 128 partitions.  Chunk c
                reads state rows 8c .. 9+8c, i.e. it depends on the previous
                step's adds for quarters c-1, c, c+1 only (plus the halo
                copies at the strip edges), which is what lets the stencil
                overlap the compute waves."""
                cur = S[g % 2]
                ft = get_ft(g)
                ftf = ft[:, :, :].rearrange("p a b -> p (a b)")
                s16f = cur[:, :, :].rearrange("p a b -> p (a b)")
                s16mf = s16m[:, :, :].rearrange("p a b -> p (a b)")
                sxf = SX[:, :, :].rearrange("p a b -> p (a b)")
                syf = SY[:, :, :].rearrange("p a b -> p (a b)")
                sxmf = SXm[:, :, :].rearrange("p a b -> p (a b)")
                symf = SYm[:, :, :].rearrange("p a b -> p (a b)")

                ra = 8 * c
                # masked-output flat window for this chunk
                lo = (ra + 1) * WP
                hi = (ra + 9) * WP
                # vertical [1,1]: T1 rows ra..ra+8
                t1a, t1b = ra * WP, min((ra + 9) * WP, SFLAT - WP)
                tt(ENG["t1"], ADD, T1[:, t1a:t1b],
                   s16f[:, t1a:t1b], s16f[:, t1a + WP:t1b + WP])
                # vertical [1,2,1]: T2 rows ra+1..ra+8
                tt(ENG["t2"], ADD, T2[:, lo:hi],
                   T1[:, lo - WP:hi - WP], T1[:, lo:hi])
                # horizontal [1,1]: T3 rows ra..ra+9
                t3a = ra * WP
                t3b = min((ra + 10) * WP, SFLAT - 1)
                tt(ENG["t3"], ADD, T3[:, t3a:t3b],
                   s16f[:, t3a:t3b], s16f[:, t3a + 1:t3b + 1])
                # horizontal [1,2,1]: T4 rows ra..ra+9
                t4a = max(t3a, 1)
                tt(ENG["t4"], ADD, T4[:, t4a:t3b],
                   T3[:, t4a - 1:t3b - 1], T3[:, t4a:t3b])
                # diffs + fire masking
                tt(ENG["subx"], SUB, sxf[:, lo:hi],
                   T2[:, lo + 1:hi + 1], T2[:, lo - 1:hi - 1])
                tt(ENG["suby"], SUB, syf[:, lo:hi],
                   T4[:, lo + WP:hi + WP], T4[:, lo - WP:hi - WP])
                tt(ENG["mask"], MULT, s16mf[:, lo:hi], s16f[:, lo:hi],
                   ftf[:, lo:hi])
                tt(ENG["mask"], MULT, sxmf[:, lo:hi], sxf[:, lo:hi],
                   ftf[:, lo:hi])
                tt(ENG["mask"], MULT, symf[:, lo:hi], syf[:, lo:hi],
                   ftf[:, lo:hi])

            def emit_tile(g, i, hf, rb):
                p0 = i * 2 * CB + hf * CB
                r0 = 1 + rb * ROWS_PER_TILE
                ti = (i * 2 + hf) * N_RB + rb
                perc = percp.tile([2 * CB, ROWS_PER_TILE, W], bf16, name="perc")
                tcopy(ENG["gather"][2 * ti],
                      perc[0:NCH],
                      SXm[p0:p0 + NCH, r0:r0 + ROWS_PER_TILE, CL:CL + W])
                tcopy(ENG["gather"][2 * ti + 1],
                      perc[CB:CB + NCH],
                      SYm[p0:p0 + NCH, r0:r0 + ROWS_PER_TILE, CL:CL + W])
                percf = perc[:, :, :].rearrange("p a b -> p (a b)")

                rhs_s = s16m[p0:p0 + NCH + 1, r0:r0 + ROWS_PER_TILE, CL:CL + W]

                h1 = ph1.tile([HIDDEN, 2 * NPIX_TILE], f32, name="h1")
                for half in range(2):
                    c0 = half * HIDDEN
                    o = h1[:, half * NPIX_TILE:(half + 1) * NPIX_TILE]
                    nc.tensor.matmul(
                        o, w1all[p0:p0 + NCH + 1, g % steps, c0:c0 + HIDDEN],
                        rhs_s, start=True, stop=False,
                        tile_position=(p0, 0))
                    nc.tensor.matmul(o, w1xy[:, c0:c0 + HIDDEN],
                                     percf, start=False, stop=True)

                # single bias-free plain-leaky activation over both halves
                # (w1/b1 are prescaled by H1SCALE on the host so the fp8
                # output lands in e4m3's normal range); output is the two
                # K-planes of the fp8 DoubleRow rhs.
                h1s = actp.tile([HIDDEN, 2, NPIX_TILE], fp8, name="h1s")
                h1sf = h1s[:, :, :].rearrange("p a b -> p (a b)")
                l1e = ENG["l1act"][ti]
                if l1e == "scalar":
                    nc.scalar.activation(out=h1sf, in_=h1[:], func=LR,
                                         bias=0.0, scale=1.0, alpha=0.01)
                else:
                    eng(l1e).scalar_tensor_tensor(
                        out=h1sf, in0=h1[:], scalar=0.01, in1=h1[:],
                        op0=MULT, op1=MAX)

                h2 = ph2.tile([HIDDEN, NPIX_TILE], f32, name="h2")
                nc.tensor.matmul(h2[:], w2dr[:], h1s[:],
                                 perf_mode=DR, start=True, stop=True)

                h2s = actp.tile([HIDDEN, NPIX_TILE], bf16, name="h2s")
                le = ENG["l2act"][ti]
                if le == "scalar":
                    nc.scalar.activation(out=h2s[:], in_=h2[:], func=LR,
                                         bias=0.0, scale=1.0, alpha=0.01)
                else:
                    # leaky(x) = max(0.01*x, x)
                    eng(le).scalar_tensor_tensor(
                        out=h2s[:], in0=h2[:], scalar=0.01, in1=h2[:],
                        op0=MULT, op1=MAX)

                dxp = pdx.tile([NCH, ROWS_PER_TILE, W], f32, name="dx")
                nc.tensor.matmul(
                    dxp[:, :, :].rearrange("p a b -> p (a b)"),
                    w3T[:], h2s[:], start=True, stop=True)
                return dxp

            def emit_rb_wave(g, rb):
                """One compute wave (all 4 blocks) for row-quarter rb."""
                cur, nxt = S[g % 2], S[(g + 1) % 2]
                r0 = 1 + rb * ROWS_PER_TILE
                for i in range(B_LOC):
                    for hf in range(2):
                        p0 = i * 2 * CB + hf * CB
                        dxp = emit_tile(g, i, hf, rb)
                        ti = (i * 2 + hf) * N_RB + rb
                        # state add on DVE (gpsimd cannot read PSUM)
                        nc.vector.tensor_add(
                            out=nxt[p0:p0 + NCH,
                                    r0:r0 + ROWS_PER_TILE, CL:CL + W],
                            in0=cur[p0:p0 + NCH,
                                    r0:r0 + ROWS_PER_TILE, CL:CL + W],
                            in1=dxp[:])
                he = eng(ENG["halo"])
                if rb == N_RB - 1:
                    # half1 strip-row 0 <- half0 interior row 32 (in rb3)
                    for i in range(B_LOC):
                        ip = i * 2 * CB
                        he.tensor_copy(out=nxt[ip + CB:ip + CB + NCH, 0:1, :],
                                       in_=nxt[ip:ip + NCH, 32:33, :])
                if rb == 0:
                    # half0 strip-row 33 <- half1 interior row 1 (in rb0)
                    for i in range(B_LOC):
                        ip = i * 2 * CB
                        he.tensor_copy(out=nxt[ip:ip + NCH, 33:34, :],
                                       in_=nxt[ip + CB:ip + CB + NCH, 1:2, :])

            # warm the PE p-state during the init DMAs
            for _w in range(24):
                wp = ph1.tile([HIDDEN, 2 * NPIX_TILE], f32, name="h1")
                nc.tensor.matmul(wp[:, 0:HIDDEN], w1all[:, 0, 0:HIDDEN],
                                 w1all[:, 0, 0:HIDDEN], start=True, stop=True)

            # Wave order: each chunk c of step g needs quarters {c-1,c,c+1}
            # of step g-1's adds (plus halo edges).  With this order, the
            # first wave's chunk only needs quarters finished one wave
            # early, so the stencil hides inside the compute stream.
            ORDER = [1, 0, 2, 3]
            TOT = steps * repeat
            for g in range(TOT):
                if g + 1 < TOT:
                    get_ft(g + 1)   # prefetch next step's fire strip
                for rb in ORDER:
                    emit_chunk(g, rb)
                    emit_rb_wave(g, rb)

            nc.sync.dma_start(out=out_d[:], in_=S[TOT % 2][:])

    orig = nc.to_json_bytes
    nc.to_json_bytes = lambda: _fix_bir_waits(orig())
    return nc


_CACHE = {}


def _get_nc(steps: int, repeat: int = 1):
    key = (steps, repeat)
    if key not in _CACHE:
        _CACHE[key] = _build(steps, repeat)
    return _CACHE[key]


def _prep_inputs(x, w1, b1, w2, w3, steps):
    """Host-side input preparation; returns per-core input maps."""
    x = np.asarray(x, np.float32)
    w1 = np.asarray(w1, np.float32)
    b1 = np.asarray(b1, np.float32)
    w2 = np.asarray(w2, np.float32)
    w3 = np.asarray(w3, np.float32)

    hid, fires = _host_rng(steps)

    bf = ml_dtypes.bfloat16
    f8 = ml_dtypes.float8_e4m3

    # full padded state [B, 32, 66->SR? rows 0..65 per image] with the
    # channel blocks padded to 32; row 29 is the constant ones row.
    state0 = np.zeros((B, CB, H + 2, WP), np.float32)
    state0[:, :NUM_IMG, 1:1 + H, CL:CL + W] = x
    state0[:, NUM_IMG:NUM_IMG + NUM_HID, 1:1 + H, CL:CL + W] = hid
    state0[:, NCH, :, :] = 1.0          # ones row (bias lane)

    # L1 s-tap weights, quadruplicated at bases 0/32/64/96; row 29 of
    # each block is the per-step bias row b_eff(t) (ones-row trick), so the
    # table is [128, steps, 256] and each step's slab is used as lhsT
    # directly.  Prescale by H1SCALE (a power of two -- lossless in bf16)
    # so the L1 activation output lands in fp8e4's normal range.
    nb = max(steps, 1)
    w1all = np.zeros((128, nb, 2 * HIDDEN), np.float32)
    for t in range(steps):
        beff = H1SCALE * (b1 + w1[:, 3 * NCH] * (np.float32(t) / np.float32(100.0)))
        for b0 in (0, 32, 64, 96):
            w1all[b0:b0 + NCH, t] = H1SCALE * w1[:, 0:NCH].T
            w1all[b0 + NCH, t] = beff
    w1all = w1all.astype(bf)
    w1xy = np.zeros((2 * CB, 2 * HIDDEN), np.float32)
    w1xy[0:NCH] = H1SCALE * w1[:, NCH:2 * NCH].T / 8.0
    w1xy[CB:CB + NCH] = H1SCALE * w1[:, 2 * NCH:3 * NCH].T / 8.0
    w1xy = w1xy.astype(bf)

    # L2 as fp8 DoubleRow planes: w2dr[k, i, m] = W2SCALE * w2[m, 128i+k]
    w2dr = np.zeros((HIDDEN, 2, HIDDEN), np.float32)
    w2dr[:, 0, :] = W2SCALE * w2[:, :HIDDEN].T
    w2dr[:, 1, :] = W2SCALE * w2[:, HIDDEN:].T
    w2dr = w2dr.astype(f8)

    w3Tf = w3.T.copy() / (W2SCALE * H1SCALE)
    w3Tf[:, :NUM_IMG] = 0.0      # image channels are immutable
    w3T = np.ascontiguousarray(w3Tf).astype(bf)

    # fire in strip layout [steps, 128, SR, WP]
    fireS = np.zeros((nb, B, 2, SR, WP), np.float32)
    for hf in range(2):
        r0 = hf * CB            # image row of strip row 1 is r0, strip rows
        fireS[:, :, hf, 1:33, CL:CL + W] = fires[:nb, :, r0:r0 + CB, :]
    fireS = fireS.astype(bf)

    in_maps = []
    for c in range(N_CORES):
        imgs = slice(c * B_LOC, (c + 1) * B_LOC)
        sc = state0[imgs]                      # [B_LOC, 32, 66, WP]
        s0 = np.stack([sc[:, :, 0:SR, :], sc[:, :, 32:32 + SR, :]], axis=2)
        s0 = s0.transpose(0, 2, 1, 3, 4).reshape(B_LOC * 2 * CB, SR, WP)
        fc = fireS[:, imgs]                    # [steps, B_LOC, 2, SR, WP]
        fc = np.broadcast_to(fc[:, :, :, None],
                             (nb, B_LOC, 2, CB, SR, WP))
        fc = fc.reshape(nb, 128, SR, WP)
        in_maps.append({
            "s0": np.ascontiguousarray(s0).astype(bf),
            "fire": np.ascontiguousarray(fc),
            "w1all": w1all, "w1xy": w1xy,
            "w2dr": w2dr, "w3T": w3T,
        })
    return in_maps


def _softmax(x):
    m = x.max(axis=-1, keepdims=True)
    e = np.exp(x - m)
    return e / e.sum(axis=-1, keepdims=True)


def _epilogue(results):
    logits = np.zeros((B, NUM_OUT), np.float32)
    for c, res in enumerate(results):
        so = res["sout"].astype(np.float32).reshape(B_LOC, 2, CB, SR, WP)
        cls = so[:, :, NUM_IMG + NUM_HID:NCH, 1:33, CL:CL + W]
        logits[c * B_LOC:(c + 1) * B_LOC] = cls.mean(axis=(1, 3, 4))
    return _softmax(logits).astype(np.float32)


def _run(trace=False, repeat=1, _in_maps=None, **inputs):
    from concourse.bass_utils import run_bass_kernel_spmd
    steps = int(inputs["steps"])
    if steps == 0:
        return _softmax(np.zeros((B, NUM_OUT), np.float32)), None
    in_maps = _in_maps
    if in_maps is None:
        in_maps = _prep_inputs(inputs["x"], inputs["w1"], inputs["b1"],
                               inputs["w2"], inputs["w3"], steps)
    nc = _get_nc(steps, repeat)
    r = run_bass_kernel_spmd(nc, in_maps, core_ids=list(range(N_CORES)),
                             trace=trace)
    return _epilogue(r.results), r.exec_time_ns


def predicted_exec_ns(steps: int = 20) -> float:
    """Cost-model (TimelineSim) estimate of on-device execution time."""
    from concourse.timeline_sim import TimelineSim
    nc = _build(int(steps))
    return TimelineSim(nc, trace=False).simulate()


def kernel(**inputs) -> np.ndarray:
    out, _ = _run(trace=False, **inputs)
    return out
